# revision 4
# baseline (speedup 1.0000x reference)
"""Trainium2 Bass kernel for nn_MessagePassingBlock (GNN message passing).

Math (reference):
    h     = x @ W_msg                       # (N, D)
    msg   = (h[source] + rel_bias[edge_type]) * edge_weights[:, None]
    delta = segment_sum(msg, target, N)     # (N, D)
    out   = relu(x @ W_self + delta + b)

Distribution: target-sharded across 8 cores (no collectives). Core c owns
nodes [c*12544, (c+1)*12544); every edge lives on its target's core.

Per-core algorithm (all matmul-based, no per-edge transposes):
  For each 128-node target block b, accumulate over that block's edges
  (chunks of 128 edges, gathered via batched SWDGE dma_gather from a bf16
  mirror of x):
      sT[k, j] += sum_e xg[e, k] * w_e * [tgt_e == j]      (PE, bf16)
      CT[r, j] += sum_e [et_e == r] * w_e * [tgt_e == j]   (PE, bf16)
  then
      out_b = relu(sT^T @ W_msg + CT^T @ rel_bias + x_b @ W_self + b)
  The onehot operands are built with single fused DVE tensor_scalar ops.
  Edge weights are folded into the target-onehot; padding edges carry w=0
  so they contribute exactly zero (self-masking).

Gather: x is split into 4 row subtables (<=32767 rows, int16 indices);
one dma_gather instruction per (superblock of 14 blocks, subtable), spread
across the 4 SWDGE queues.
"""

import functools
import math

import numpy as np
import ml_dtypes

NUM_NODES = 100000
D = 128
NUM_REL = 8
N_CORES = 8
NODES_PER_CORE = 12544          # 98 blocks of 128
NBLK = NODES_PER_CORE // 128    # 98
SB_BLOCKS = 14                  # blocks per superblock
N_SB = NBLK // SB_BLOCKS        # 7
N_SUBT = 4
SUBT_ROWS = 25000               # rows per gather subtable

_kernel_cache = {}


def _build_and_compile(c_bt_key, nchunks_sbt, chunk_plan):
    """Build + compile the SPMD Bass kernel for a given static chunk layout.

    nchunks_sbt: [N_SB][N_SUBT] -> number of 128-edge chunks in that
        gather instruction.
    chunk_plan: [NBLK] -> list of (t, slot_in_sbt_tile, global_chunk_id)
        in processing order for that block.
    """
    import concourse.bacc as bacc
    import concourse.tile as tile
    import concourse.mybir as mybir
    from concourse.masks import make_identity

    NC_TOT = sum(sum(row) for row in nchunks_sbt)

    nc = bacc.Bacc(
        "TRN2",
        target_bir_lowering=False,
        debug=False,
        num_devices=N_CORES,
        num_swdge_queues=4,
    )
    f32 = mybir.dt.float32
    bf16 = mybir.dt.bfloat16
    i16 = mybir.dt.int16

    xbf = nc.dram_tensor("xbf", [NUM_NODES, D], bf16, kind="ExternalInput")
    x_shard = nc.dram_tensor("x_shard", [NODES_PER_CORE, D], f32, kind="ExternalInput")
    w_msg = nc.dram_tensor("w_msg", [D, D], f32, kind="ExternalInput")
    w_self = nc.dram_tensor("w_self", [D, D], f32, kind="ExternalInput")
    rel_bias = nc.dram_tensor("rel_bias", [NUM_REL, D], f32, kind="ExternalInput")
    bvec = nc.dram_tensor("bvec", [1, D], f32, kind="ExternalInput")
    # gather indices, already 16-partition-wrapped + replicated to 128
    n_idx_cols = sum(n * 128 // 16 for row in nchunks_sbt for n in row)
    gidx = nc.dram_tensor("gidx", [128, n_idx_cols], i16, kind="ExternalInput")
    tgt_meta = nc.dram_tensor("tgt_meta", [128, NC_TOT], f32, kind="ExternalInput")
    w_meta = nc.dram_tensor("w_meta", [128, NC_TOT], f32, kind="ExternalInput")
    et_meta = nc.dram_tensor("et_meta", [128, NC_TOT], f32, kind="ExternalInput")
    out_d = nc.dram_tensor("out", [NODES_PER_CORE, D], f32, kind="ExternalOutput")

    with tile.TileContext(nc) as tc:
        with tc.tile_pool(name="const", bufs=1) as cpool, tc.tile_pool(
            name="meta", bufs=1
        ) as mpool, tc.tile_pool(name="gath", bufs=2) as gpool, tc.tile_pool(
            name="oh", bufs=6
        ) as ohpool, tc.tile_pool(name="blk", bufs=3) as bpool, tc.tile_pool(
            name="ps", bufs=2, space="PSUM"
        ) as pspool, tc.tile_pool(name="pso", bufs=2, space="PSUM") as psopool:
            # ---- constants ----
            ident = cpool.tile([128, 128], f32)
            make_identity(nc, ident[:])
            iota128 = cpool.tile([128, 128], bf16)
            nc.gpsimd.iota(
                iota128[:], pattern=[[1, 128]], base=0, channel_multiplier=0,
                allow_small_or_imprecise_dtypes=True,
            )
            iota8 = cpool.tile([128, NUM_REL], bf16)
            nc.gpsimd.iota(
                iota8[:], pattern=[[1, NUM_REL]], base=0, channel_multiplier=0,
                allow_small_or_imprecise_dtypes=True,
            )
            wmsg_f = cpool.tile([128, D], f32)
            nc.sync.dma_start(out=wmsg_f[:], in_=w_msg.ap())
            wmsg_b = cpool.tile([128, D], bf16)
            nc.vector.tensor_copy(out=wmsg_b[:], in_=wmsg_f[:])
            wself_f = cpool.tile([128, D], f32)
            nc.sync.dma_start(out=wself_f[:], in_=w_self.ap())
            rb_f = cpool.tile([NUM_REL, D], f32)
            nc.sync.dma_start(out=rb_f[:], in_=rel_bias.ap())
            rb_b = cpool.tile([NUM_REL, D], bf16)
            nc.vector.tensor_copy(out=rb_b[:], in_=rb_f[:])
            b_bcast = cpool.tile([128, D], f32)
            nc.sync.dma_start(out=b_bcast[:], in_=bvec.ap().to_broadcast([128, D]))

            # ---- edge metadata (one DMA each) ----
            tgt_t = mpool.tile([128, NC_TOT], f32)
            nc.sync.dma_start(out=tgt_t[:], in_=tgt_meta.ap())
            w_t = mpool.tile([128, NC_TOT], f32)
            nc.sync.dma_start(out=w_t[:], in_=w_meta.ap())
            et_t = mpool.tile([128, NC_TOT], f32)
            nc.sync.dma_start(out=et_t[:], in_=et_meta.ap())
            gidx_t = mpool.tile([128, n_idx_cols], i16)
            nc.sync.dma_start(out=gidx_t[:], in_=gidx.ap())

            # precompute static offsets
            idx_off = {}
            off = 0
            for sb in range(N_SB):
                for t in range(N_SUBT):
                    idx_off[(sb, t)] = off
                    off += nchunks_sbt[sb][t] * 128 // 16

            gmax = [max(nchunks_sbt[sb][t] for sb in range(N_SB)) for t in range(N_SUBT)]

            for sb in range(N_SB):
                # ---- 4 gather instructions for this superblock ----
                gtiles = []
                for t in range(N_SUBT):
                    nck = nchunks_sbt[sb][t]
                    gt = gpool.tile([128, gmax[t] * 128], bf16, tag=f"g{t}")
                    if nck > 0:
                        n = nck * 128
                        base = t * SUBT_ROWS
                        rows = min(SUBT_ROWS, NUM_NODES - base)
                        io = idx_off[(sb, t)]
                        nc.gpsimd.dma_gather(
                            out_ap=gt[:, : nck * 128].rearrange(
                                "p (c r) -> p c r", r=128
                            ),
                            in_ap=xbf.ap()[base : base + rows, :],
                            idxs_ap=gidx_t[:, io : io + n // 16],
                            num_idxs=n,
                            num_idxs_reg=n,
                            elem_size=D,
                            single_packet=False,
                            queue_num=t,
                        )
                    gtiles.append(gt)

                for bi in range(SB_BLOCKS):
                    blk = sb * SB_BLOCKS + bi
                    plan = chunk_plan[blk]
                    assert plan, f"block {blk} has no chunks"
                    sT = pspool.tile([128, 128], f32, tag="sT")
                    cT = pspool.tile([NUM_REL, 128], f32, tag="cT")
                    nchunk = len(plan)
                    for ci, (t, slot, gchunk) in enumerate(plan):
                        ohw = ohpool.tile([128, 128], bf16, tag="ohw")
                        nc.vector.tensor_scalar(
                            out=ohw[:],
                            in0=iota128[:],
                            scalar1=tgt_t[:, gchunk : gchunk + 1],
                            scalar2=w_t[:, gchunk : gchunk + 1],
                            op0=mybir.AluOpType.is_equal,
                            op1=mybir.AluOpType.mult,
                        )
                        ohe = ohpool.tile([128, NUM_REL], bf16, tag="ohe")
                        nc.vector.tensor_scalar(
                            out=ohe[:],
                            in0=iota8[:],
                            scalar1=et_t[:, gchunk : gchunk + 1],
                            scalar2=None,
                            op0=mybir.AluOpType.is_equal,
                        )
                        xg = gtiles[t][:, slot * 128 : (slot + 1) * 128]
                        nc.tensor.matmul(
                            out=sT[:], lhsT=xg, rhs=ohw[:],
                            start=(ci == 0), stop=(ci == nchunk - 1),
                        )
                        nc.tensor.matmul(
                            out=cT[:], lhsT=ohe[:], rhs=ohw[:],
                            start=(ci == 0), stop=(ci == nchunk - 1),
                        )

                    # ---- block epilogue ----
                    sT_sb = bpool.tile([128, 128], bf16, tag="sTsb")
                    nc.vector.tensor_copy(out=sT_sb[:], in_=sT[:])
                    cT_sb = bpool.tile([NUM_REL, 128], bf16, tag="cTsb")
                    nc.vector.tensor_copy(out=cT_sb[:], in_=cT[:])
                    x_blk = bpool.tile([128, D], f32, tag="xblk")
                    nc.sync.dma_start(
                        out=x_blk[:], in_=x_shard.ap()[blk * 128 : (blk + 1) * 128, :]
                    )
                    xT_ps = psopool.tile([128, 128], f32, tag="xT")
                    nc.tensor.transpose(out=xT_ps[:], in_=x_blk[:], identity=ident[:])
                    xT_sb = bpool.tile([128, 128], f32, tag="xTsb")
                    nc.vector.tensor_copy(out=xT_sb[:], in_=xT_ps[:])

                    acc = psopool.tile([128, D], f32, tag="acc")
                    nc.tensor.matmul(
                        out=acc[:], lhsT=sT_sb[:], rhs=wmsg_b[:],
                        start=True, stop=False,
                    )
                    nc.tensor.matmul(
                        out=acc[:], lhsT=cT_sb[:], rhs=rb_b[:],
                        start=False, stop=False,
                    )
                    nc.tensor.matmul(
                        out=acc[:], lhsT=xT_sb[:], rhs=wself_f[:],
                        start=False, stop=True,
                    )
                    o_sb = bpool.tile([128, D], f32, tag="osb")
                    nc.vector.tensor_add(out=o_sb[:], in0=acc[:], in1=b_bcast[:])
                    nc.vector.tensor_scalar_max(out=o_sb[:], in0=o_sb[:], scalar1=0.0)
                    nc.sync.dma_start(
                        out=out_d.ap()[blk * 128 : (blk + 1) * 128, :], in_=o_sb[:]
                    )

    nc.compile()
    return nc


def _prep(inputs):
    """Host-side sharding/layout. Returns (in_maps, static_key, layout)."""
    x = np.ascontiguousarray(np.asarray(inputs["x"], dtype=np.float32))
    source = np.asarray(inputs["source"]).astype(np.int64)
    target = np.asarray(inputs["target"]).astype(np.int64)
    edge_type = np.asarray(inputs["edge_type"]).astype(np.int64)
    ew = np.asarray(inputs["edge_weights"], dtype=np.float32)
    w_msg = np.ascontiguousarray(np.asarray(inputs["W_msg"], dtype=np.float32))
    rel_bias = np.ascontiguousarray(np.asarray(inputs["rel_bias"], dtype=np.float32))
    w_self = np.ascontiguousarray(np.asarray(inputs["W_self"], dtype=np.float32))
    b = np.asarray(inputs["b"], dtype=np.float32).reshape(1, D)

    n = x.shape[0]
    assert n == NUM_NODES

    xbf = x.astype(ml_dtypes.bfloat16)

    core = target // NODES_PER_CORE
    tgt_local = target - core * NODES_PER_CORE
    blk = tgt_local >> 7
    tgt_in_blk = tgt_local & 127
    subt = source // SUBT_ROWS
    src_local = source - subt * SUBT_ROWS

    # per (core, blk, subtable) edge index lists
    # order edges by (core, blk, subt) with a stable sort
    key = ((core * NBLK + blk) * N_SUBT + subt).astype(np.int64)
    order = np.argsort(key, kind="stable")
    key_s = key[order]
    # group boundaries
    uniq, starts = np.unique(key_s, return_index=True)
    counts = np.diff(np.append(starts, key_s.shape[0]))

    cnt = np.zeros((N_CORES, NBLK, N_SUBT), dtype=np.int64)
    ci = uniq // (NBLK * N_SUBT)
    bi = (uniq // N_SUBT) % NBLK
    ti = uniq % N_SUBT
    cnt[ci, bi, ti] = counts

    # static chunk capacity per (blk, subtable): max over cores
    c_bt = np.ceil(cnt.max(axis=0) / 128).astype(np.int64)  # (NBLK, N_SUBT)
    # ensure every block has at least one chunk
    empty = c_bt.sum(axis=1) == 0
    c_bt[empty, 0] = 1

    nchunks_sbt = [
        [int(c_bt[sb * SB_BLOCKS : (sb + 1) * SB_BLOCKS, t].sum()) for t in range(N_SUBT)]
        for sb in range(N_SB)
    ]
    NC_TOT = int(c_bt.sum())

    # global chunk ids: order is (sb, t, blk-within-sb, chunk)
    gchunk_of = np.zeros((NBLK, N_SUBT), dtype=np.int64)  # first chunk id
    slot_of = np.zeros((NBLK, N_SUBT), dtype=np.int64)    # first slot in (sb,t) tile
    g = 0
    for sb in range(N_SB):
        for t in range(N_SUBT):
            s = 0
            for bi2 in range(SB_BLOCKS):
                bb = sb * SB_BLOCKS + bi2
                gchunk_of[bb, t] = g
                slot_of[bb, t] = s
                g += int(c_bt[bb, t])
                s += int(c_bt[bb, t])
    assert g == NC_TOT

    chunk_plan = []
    for bb in range(NBLK):
        plan = []
        for t in range(N_SUBT):
            for c in range(int(c_bt[bb, t])):
                plan.append((t, int(slot_of[bb, t] + c), int(gchunk_of[bb, t] + c)))
        chunk_plan.append(plan)

    n_idx_cols = sum(nc_ * 128 // 16 for row in nchunks_sbt for nc_ in row)

    # build per-core tensors
    in_maps = []
    # offsets of edge groups in the sorted edge array, per core
    start_of = {}
    for u, s0, c0 in zip(uniq, starts, counts):
        start_of[int(u)] = (int(s0), int(c0))

    for c in range(N_CORES):
        gidx = np.zeros((128, n_idx_cols), dtype=np.int16)
        tgt_m = np.zeros((128, NC_TOT), dtype=np.float32)
        w_m = np.zeros((128, NC_TOT), dtype=np.float32)
        et_m = np.zeros((128, NC_TOT), dtype=np.float32)

        icol = 0
        for sb in range(N_SB):
            for t in range(N_SUBT):
                nck = nchunks_sbt[sb][t]
                if nck == 0:
                    continue
                nslots = nck * 128
                idxs = np.zeros(nslots, dtype=np.int16)
                for bi2 in range(SB_BLOCKS):
                    bb = sb * SB_BLOCKS + bi2
                    k = (c * NBLK + bb) * N_SUBT + t
                    s0, n_e = start_of.get(k, (0, 0))
                    sl0 = int(slot_of[bb, t]) * 128 - int(slot_of[sb * SB_BLOCKS, t]) * 128
                    g0 = int(gchunk_of[bb, t])
                    if n_e:
                        eids = order[s0 : s0 + n_e]
                        idxs[sl0 : sl0 + n_e] = src_local[eids].astype(np.int16)
                        # meta: chunk-major [128 partitions]
                        for cc in range(int(c_bt[bb, t])):
                            lo = cc * 128
                            hi = min(n_e, lo + 128)
                            if hi <= lo:
                                break
                            ecol = eids[lo:hi]
                            gc = g0 + cc
                            npart = hi - lo
                            tgt_m[:npart, gc] = tgt_in_blk[ecol].astype(np.float32)
                            w_m[:npart, gc] = ew[ecol].astype(np.float32)
                            et_m[:npart, gc] = edge_type[ecol].astype(np.float32)
                # wrap idxs: element j -> partition j%16, col j//16; replicate x8
                wrapped = idxs.reshape(nslots // 16, 16).T  # (16, nslots/16)
                gidx[:, icol : icol + nslots // 16] = np.tile(wrapped, (8, 1))
                icol += nslots // 16
        assert icol == n_idx_cols

        xs = np.zeros((NODES_PER_CORE, D), dtype=np.float32)
        lo = c * NODES_PER_CORE
        hi = min(lo + NODES_PER_CORE, NUM_NODES)
        xs[: hi - lo] = x[lo:hi]

        in_maps.append(
            {
                "xbf": xbf,
                "x_shard": xs,
                "w_msg": w_msg,
                "w_self": w_self,
                "rel_bias": rel_bias,
                "bvec": b,
                "gidx": gidx,
                "tgt_meta": tgt_m,
                "w_meta": w_m,
                "et_meta": et_m,
            }
        )

    static_key = tuple(c_bt.flatten().tolist())
    return in_maps, static_key, (nchunks_sbt, chunk_plan)


def kernel(**inputs) -> np.ndarray:
    from concourse import bass_utils

    in_maps, static_key, (nchunks_sbt, chunk_plan) = _prep(inputs)

    nc = _kernel_cache.get(static_key)
    if nc is None:
        nc = _build_and_compile(static_key, nchunks_sbt, chunk_plan)
        _kernel_cache[static_key] = nc

    res = bass_utils.run_bass_kernel_spmd(
        nc, in_maps, core_ids=list(range(N_CORES))
    )
    parts = [res.results[c]["out"] for c in range(N_CORES)]
    full = np.concatenate(parts, axis=0)[:NUM_NODES]
    return full.astype(np.float32)


# revision 6
# speedup vs baseline: 1.2158x; 1.2158x over previous
"""Trainium2 Bass kernel for nn_MessagePassingBlock (GNN message passing).

Math (reference):
    h     = x @ W_msg                       # (N, D)
    msg   = (h[source] + rel_bias[edge_type]) * edge_weights[:, None]
    delta = segment_sum(msg, target, N)     # (N, D)
    out   = relu(x @ W_self + delta + b)

Distribution: target-sharded across 8 cores (no collectives). Core c owns
nodes [c*12544, (c+1)*12544); every edge lives on its target's core.

Per-core algorithm (all matmul-based, no per-edge transposes):
  For each 128-node target block b, accumulate over that block's edges
  (chunks of 128 edges, gathered via batched SWDGE dma_gather from a bf16
  mirror of x):
      sT[k, j] += sum_e xg[e, k] * w_e * [tgt_e == j]      (PE, bf16)
      CT[r, j] += sum_e [et_e == r] * w_e * [tgt_e == j]   (PE, bf16)
  then
      out_b = relu(sT^T @ W_msg + CT^T @ rel_bias + x_b @ W_self + b)
  The onehot operands are built with single fused DVE tensor_scalar ops.
  Edge weights are folded into the target-onehot; padding edges carry w=0
  so they contribute exactly zero (self-masking).

Gather: x is split into 4 row subtables (<=32767 rows, int16 indices);
one dma_gather instruction per (superblock of 14 blocks, subtable), spread
across the 4 SWDGE queues.
"""

import functools
import math

import numpy as np
import ml_dtypes

NUM_NODES = 100000
D = 128
NUM_REL = 8
N_CORES = 8
NODES_PER_CORE = 12544          # 98 blocks of 128
NBLK = NODES_PER_CORE // 128    # 98
SB_BLOCKS = 14                  # blocks per superblock
N_SB = NBLK // SB_BLOCKS        # 7
N_SUBT = 4
SUBT_ROWS = 25000               # rows per gather subtable

_kernel_cache = {}


def _build_and_compile(c_bt_key, nchunks_sbt, chunk_plan):
    """Build + compile the SPMD Bass kernel for a given static chunk layout.

    nchunks_sbt: [N_SB][N_SUBT] -> number of 128-edge chunks in that
        gather instruction.
    chunk_plan: [NBLK] -> list of (t, slot_in_sbt_tile, global_chunk_id)
        in processing order for that block.
    """
    import concourse.bacc as bacc
    import concourse.tile as tile
    import concourse.mybir as mybir
    from concourse.masks import make_identity

    NC_TOT = sum(sum(row) for row in nchunks_sbt)

    nc = bacc.Bacc(
        "TRN2",
        target_bir_lowering=False,
        debug=False,
        num_devices=N_CORES,
        num_swdge_queues=4,
    )
    f32 = mybir.dt.float32
    bf16 = mybir.dt.bfloat16
    i16 = mybir.dt.int16

    xbf = nc.dram_tensor("xbf", [NUM_NODES, D], bf16, kind="ExternalInput")
    x_shard = nc.dram_tensor("x_shard", [NODES_PER_CORE, D], f32, kind="ExternalInput")
    w_msg = nc.dram_tensor("w_msg", [D, D], f32, kind="ExternalInput")
    w_self = nc.dram_tensor("w_self", [D, D], f32, kind="ExternalInput")
    rel_bias = nc.dram_tensor("rel_bias", [NUM_REL, D], f32, kind="ExternalInput")
    bvec = nc.dram_tensor("bvec", [1, D], f32, kind="ExternalInput")
    # gather indices, already 16-partition-wrapped + replicated to 128
    n_idx_cols = sum(n * 128 // 16 for row in nchunks_sbt for n in row)
    gidx = nc.dram_tensor("gidx", [128, n_idx_cols], i16, kind="ExternalInput")
    ohw_meta = nc.dram_tensor("ohw_meta", [128, NC_TOT * 128], bf16, kind="ExternalInput")
    ohe_meta = nc.dram_tensor("ohe_meta", [128, NC_TOT * NUM_REL], bf16, kind="ExternalInput")
    out_d = nc.dram_tensor("out", [NODES_PER_CORE, D], f32, kind="ExternalOutput")

    with tile.TileContext(nc) as tc:
        with tc.tile_pool(name="const", bufs=1) as cpool, tc.tile_pool(
            name="meta", bufs=1
        ) as mpool, tc.tile_pool(name="gath", bufs=2) as gpool, tc.tile_pool(
            name="oh", bufs=6
        ) as ohpool, tc.tile_pool(name="blk", bufs=3) as bpool, tc.tile_pool(
            name="ps", bufs=2, space="PSUM"
        ) as pspool, tc.tile_pool(name="pso", bufs=2, space="PSUM") as psopool:
            # ---- constants ----
            ident = cpool.tile([128, 128], f32)
            make_identity(nc, ident[:])
            wmsg_f = cpool.tile([128, D], f32)
            nc.sync.dma_start(out=wmsg_f[:], in_=w_msg.ap())
            wmsg_b = cpool.tile([128, D], bf16)
            nc.vector.tensor_copy(out=wmsg_b[:], in_=wmsg_f[:])
            wself_f = cpool.tile([128, D], f32)
            nc.sync.dma_start(out=wself_f[:], in_=w_self.ap())
            rb_f = cpool.tile([NUM_REL, D], f32)
            nc.sync.dma_start(out=rb_f[:], in_=rel_bias.ap())
            rb_b = cpool.tile([NUM_REL, D], bf16)
            nc.vector.tensor_copy(out=rb_b[:], in_=rb_f[:])
            b_bcast = cpool.tile([128, D], f32)
            nc.sync.dma_start(out=b_bcast[:], in_=bvec.ap().to_broadcast([128, D]))

            # ---- gather indices (one DMA) ----
            gidx_t = mpool.tile([128, n_idx_cols], i16)
            nc.sync.dma_start(out=gidx_t[:], in_=gidx.ap())

            # precompute static offsets
            idx_off = {}
            off = 0
            for sb in range(N_SB):
                for t in range(N_SUBT):
                    idx_off[(sb, t)] = off
                    off += nchunks_sbt[sb][t] * 128 // 16

            gmax = [max(nchunks_sbt[sb][t] for sb in range(N_SB)) for t in range(N_SUBT)]
            pos_of = {}
            _p = 0
            for _b in range(NBLK):
                pos_of[_b] = _p
                _p += len(chunk_plan[_b])

            for sb in range(N_SB):
                # ---- 4 gather instructions for this superblock ----
                gtiles = []
                for t in range(N_SUBT):
                    nck = nchunks_sbt[sb][t]
                    gt = gpool.tile([128, gmax[t] * 128], bf16, tag=f"g{t}")
                    if nck > 0:
                        n = nck * 128
                        base = t * SUBT_ROWS
                        rows = min(SUBT_ROWS, NUM_NODES - base)
                        io = idx_off[(sb, t)]
                        nc.gpsimd.dma_gather(
                            out_ap=gt[:, : nck * 128].rearrange(
                                "p (c r) -> p c r", r=128
                            ),
                            in_ap=xbf.ap()[base : base + rows, :],
                            idxs_ap=gidx_t[:, io : io + n // 16],
                            num_idxs=n,
                            num_idxs_reg=n,
                            elem_size=D,
                            single_packet=False,
                            queue_num=t,
                        )
                    gtiles.append(gt)

                for bi in range(SB_BLOCKS):
                    blk = sb * SB_BLOCKS + bi
                    plan = chunk_plan[blk]
                    assert plan, f"block {blk} has no chunks"
                    nchunk = len(plan)
                    pos0 = pos_of[blk]
                    ohw_b = ohpool.tile([128, nchunk * 128], bf16, tag="ohw")
                    nc.sync.dma_start(
                        out=ohw_b[:],
                        in_=ohw_meta.ap()[:, pos0 * 128 : (pos0 + nchunk) * 128],
                    )
                    ohe_b = ohpool.tile([128, nchunk * NUM_REL], bf16, tag="ohe")
                    nc.sync.dma_start(
                        out=ohe_b[:],
                        in_=ohe_meta.ap()[
                            :, pos0 * NUM_REL : (pos0 + nchunk) * NUM_REL
                        ],
                    )
                    sT = pspool.tile([128, 128], f32, tag="sT")
                    cT = pspool.tile([NUM_REL, 128], f32, tag="cT")
                    for ci, (t, slot, gchunk) in enumerate(plan):
                        ohw = ohw_b[:, ci * 128 : (ci + 1) * 128]
                        ohe = ohe_b[:, ci * NUM_REL : (ci + 1) * NUM_REL]
                        xg = gtiles[t][:, slot * 128 : (slot + 1) * 128]
                        nc.tensor.matmul(
                            out=sT[:], lhsT=xg, rhs=ohw,
                            start=(ci == 0), stop=(ci == nchunk - 1),
                        )
                        nc.tensor.matmul(
                            out=cT[:], lhsT=ohe, rhs=ohw,
                            start=(ci == 0), stop=(ci == nchunk - 1),
                        )

                    # ---- block epilogue ----
                    sT_sb = bpool.tile([128, 128], bf16, tag="sTsb")
                    nc.vector.tensor_copy(out=sT_sb[:], in_=sT[:])
                    cT_sb = bpool.tile([NUM_REL, 128], bf16, tag="cTsb")
                    nc.vector.tensor_copy(out=cT_sb[:], in_=cT[:])
                    x_blk = bpool.tile([128, D], f32, tag="xblk")
                    nc.sync.dma_start(
                        out=x_blk[:], in_=x_shard.ap()[blk * 128 : (blk + 1) * 128, :]
                    )
                    xT_ps = psopool.tile([128, 128], f32, tag="xT")
                    nc.tensor.transpose(out=xT_ps[:], in_=x_blk[:], identity=ident[:])
                    xT_sb = bpool.tile([128, 128], f32, tag="xTsb")
                    nc.vector.tensor_copy(out=xT_sb[:], in_=xT_ps[:])

                    acc = psopool.tile([128, D], f32, tag="acc")
                    nc.tensor.matmul(
                        out=acc[:], lhsT=sT_sb[:], rhs=wmsg_b[:],
                        start=True, stop=False,
                    )
                    nc.tensor.matmul(
                        out=acc[:], lhsT=cT_sb[:], rhs=rb_b[:],
                        start=False, stop=False,
                    )
                    nc.tensor.matmul(
                        out=acc[:], lhsT=xT_sb[:], rhs=wself_f[:],
                        start=False, stop=True,
                    )
                    o_sb = bpool.tile([128, D], f32, tag="osb")
                    nc.vector.tensor_add(out=o_sb[:], in0=acc[:], in1=b_bcast[:])
                    nc.vector.tensor_scalar_max(out=o_sb[:], in0=o_sb[:], scalar1=0.0)
                    nc.sync.dma_start(
                        out=out_d.ap()[blk * 128 : (blk + 1) * 128, :], in_=o_sb[:]
                    )

    nc.compile()
    return nc


def _prep(inputs):
    """Host-side sharding/layout. Returns (in_maps, static_key, layout)."""
    x = np.ascontiguousarray(np.asarray(inputs["x"], dtype=np.float32))
    source = np.asarray(inputs["source"]).astype(np.int64)
    target = np.asarray(inputs["target"]).astype(np.int64)
    edge_type = np.asarray(inputs["edge_type"]).astype(np.int64)
    ew = np.asarray(inputs["edge_weights"], dtype=np.float32)
    w_msg = np.ascontiguousarray(np.asarray(inputs["W_msg"], dtype=np.float32))
    rel_bias = np.ascontiguousarray(np.asarray(inputs["rel_bias"], dtype=np.float32))
    w_self = np.ascontiguousarray(np.asarray(inputs["W_self"], dtype=np.float32))
    b = np.asarray(inputs["b"], dtype=np.float32).reshape(1, D)

    n = x.shape[0]
    assert n == NUM_NODES

    xbf = x.astype(ml_dtypes.bfloat16)

    core = target // NODES_PER_CORE
    tgt_local = target - core * NODES_PER_CORE
    blk = tgt_local >> 7
    tgt_in_blk = tgt_local & 127
    subt = source // SUBT_ROWS
    src_local = source - subt * SUBT_ROWS

    # per (core, blk, subtable) edge index lists
    # order edges by (core, blk, subt) with a stable sort
    key = ((core * NBLK + blk) * N_SUBT + subt).astype(np.int64)
    order = np.argsort(key, kind="stable")
    key_s = key[order]
    # group boundaries
    uniq, starts = np.unique(key_s, return_index=True)
    counts = np.diff(np.append(starts, key_s.shape[0]))

    cnt = np.zeros((N_CORES, NBLK, N_SUBT), dtype=np.int64)
    ci = uniq // (NBLK * N_SUBT)
    bi = (uniq // N_SUBT) % NBLK
    ti = uniq % N_SUBT
    cnt[ci, bi, ti] = counts

    # static chunk capacity per (blk, subtable): max over cores
    c_bt = np.ceil(cnt.max(axis=0) / 128).astype(np.int64)  # (NBLK, N_SUBT)
    # ensure every block has at least one chunk
    empty = c_bt.sum(axis=1) == 0
    c_bt[empty, 0] = 1

    nchunks_sbt = [
        [int(c_bt[sb * SB_BLOCKS : (sb + 1) * SB_BLOCKS, t].sum()) for t in range(N_SUBT)]
        for sb in range(N_SB)
    ]
    NC_TOT = int(c_bt.sum())

    # global chunk ids: order is (sb, t, blk-within-sb, chunk)
    gchunk_of = np.zeros((NBLK, N_SUBT), dtype=np.int64)  # first chunk id
    slot_of = np.zeros((NBLK, N_SUBT), dtype=np.int64)    # first slot in (sb,t) tile
    g = 0
    for sb in range(N_SB):
        for t in range(N_SUBT):
            s = 0
            for bi2 in range(SB_BLOCKS):
                bb = sb * SB_BLOCKS + bi2
                gchunk_of[bb, t] = g
                slot_of[bb, t] = s
                g += int(c_bt[bb, t])
                s += int(c_bt[bb, t])
    assert g == NC_TOT

    chunk_plan = []
    for bb in range(NBLK):
        plan = []
        for t in range(N_SUBT):
            for c in range(int(c_bt[bb, t])):
                plan.append((t, int(slot_of[bb, t] + c), int(gchunk_of[bb, t] + c)))
        chunk_plan.append(plan)

    # position of each block's chunk run in the (block-major) onehot layout
    pos_of_blk = np.zeros(NBLK, dtype=np.int64)
    p = 0
    for bb in range(NBLK):
        pos_of_blk[bb] = p
        p += len(chunk_plan[bb])
    # gchunk -> block-major position
    pos_of_gchunk = np.zeros(NC_TOT, dtype=np.int64)
    for bb in range(NBLK):
        for i, (_t, _s, g2) in enumerate(chunk_plan[bb]):
            pos_of_gchunk[g2] = pos_of_blk[bb] + i

    n_idx_cols = sum(nc_ * 128 // 16 for row in nchunks_sbt for nc_ in row)

    # build per-core tensors
    in_maps = []
    # offsets of edge groups in the sorted edge array, per core
    start_of = {}
    for u, s0, c0 in zip(uniq, starts, counts):
        start_of[int(u)] = (int(s0), int(c0))

    for c in range(N_CORES):
        gidx = np.zeros((128, n_idx_cols), dtype=np.int16)
        ohw_m = np.zeros((128, NC_TOT * 128), dtype=ml_dtypes.bfloat16)
        ohe_m = np.zeros((128, NC_TOT * NUM_REL), dtype=ml_dtypes.bfloat16)

        icol = 0
        for sb in range(N_SB):
            for t in range(N_SUBT):
                nck = nchunks_sbt[sb][t]
                if nck == 0:
                    continue
                nslots = nck * 128
                idxs = np.zeros(nslots, dtype=np.int16)
                for bi2 in range(SB_BLOCKS):
                    bb = sb * SB_BLOCKS + bi2
                    k = (c * NBLK + bb) * N_SUBT + t
                    s0, n_e = start_of.get(k, (0, 0))
                    sl0 = int(slot_of[bb, t]) * 128 - int(slot_of[sb * SB_BLOCKS, t]) * 128
                    g0 = int(gchunk_of[bb, t])
                    if n_e:
                        eids = order[s0 : s0 + n_e]
                        idxs[sl0 : sl0 + n_e] = src_local[eids].astype(np.int16)
                        # meta: chunk-major [128 partitions]
                        for cc in range(int(c_bt[bb, t])):
                            lo = cc * 128
                            hi = min(n_e, lo + 128)
                            if hi <= lo:
                                break
                            ecol = eids[lo:hi]
                            gc = g0 + cc
                            npart = hi - lo
                            pos = int(pos_of_gchunk[gc])
                            parts = np.arange(npart)
                            ohw_m[parts, pos * 128 + tgt_in_blk[ecol]] = ew[
                                ecol
                            ].astype(ml_dtypes.bfloat16)
                            ohe_m[parts, pos * NUM_REL + edge_type[ecol]] = 1.0
                # wrap idxs: element j -> partition j%16, col j//16; replicate x8
                wrapped = idxs.reshape(nslots // 16, 16).T  # (16, nslots/16)
                gidx[:, icol : icol + nslots // 16] = np.tile(wrapped, (8, 1))
                icol += nslots // 16
        assert icol == n_idx_cols

        xs = np.zeros((NODES_PER_CORE, D), dtype=np.float32)
        lo = c * NODES_PER_CORE
        hi = min(lo + NODES_PER_CORE, NUM_NODES)
        xs[: hi - lo] = x[lo:hi]

        in_maps.append(
            {
                "xbf": xbf,
                "x_shard": xs,
                "w_msg": w_msg,
                "w_self": w_self,
                "rel_bias": rel_bias,
                "bvec": b,
                "gidx": gidx,
                "ohw_meta": ohw_m,
                "ohe_meta": ohe_m,
            }
        )

    static_key = tuple(c_bt.flatten().tolist())
    return in_maps, static_key, (nchunks_sbt, chunk_plan)


def kernel(**inputs) -> np.ndarray:
    from concourse import bass_utils

    in_maps, static_key, (nchunks_sbt, chunk_plan) = _prep(inputs)

    nc = _kernel_cache.get(static_key)
    if nc is None:
        nc = _build_and_compile(static_key, nchunks_sbt, chunk_plan)
        _kernel_cache[static_key] = nc

    res = bass_utils.run_bass_kernel_spmd(
        nc, in_maps, core_ids=list(range(N_CORES))
    )
    parts = [res.results[c]["out"] for c in range(N_CORES)]
    full = np.concatenate(parts, axis=0)[:NUM_NODES]
    return full.astype(np.float32)


# revision 8
# speedup vs baseline: 1.3697x; 1.1266x over previous
"""Trainium2 Bass kernel for nn_MessagePassingBlock (GNN message passing).

Math (reference):
    h     = x @ W_msg                       # (N, D)
    msg   = (h[source] + rel_bias[edge_type]) * edge_weights[:, None]
    delta = segment_sum(msg, target, N)     # (N, D)
    out   = relu(x @ W_self + delta + b)

Distribution: target-sharded across 8 cores (no collectives). Core c owns
nodes [c*12544, (c+1)*12544); every edge lives on its target's core.

Per-core algorithm (all matmul-based, no per-edge transposes):
  For each 128-node target block b, accumulate over that block's edges
  (chunks of 128 edges, gathered via batched SWDGE dma_gather from a bf16
  mirror of x):
      sT[k, j] += sum_e xg[e, k] * w_e * [tgt_e == j]      (PE, bf16)
      CT[r, j] += sum_e [et_e == r] * w_e * [tgt_e == j]   (PE, bf16)
  then
      out_b = relu(sT^T @ W_msg + CT^T @ rel_bias + x_b @ W_self + b)
  The onehot operands are built with single fused DVE tensor_scalar ops.
  Edge weights are folded into the target-onehot; padding edges carry w=0
  so they contribute exactly zero (self-masking).

Gather: x is split into 4 row subtables (<=32767 rows, int16 indices);
one dma_gather instruction per (superblock of 14 blocks, subtable), spread
across the 4 SWDGE queues.
"""

import functools
import math

import numpy as np
import ml_dtypes

NUM_NODES = 100000
D = 128
NUM_REL = 8
N_CORES = 8
NODES_PER_CORE = 12544          # 98 blocks of 128
NBLK = NODES_PER_CORE // 128    # 98
SB_BLOCKS = 14                  # blocks per superblock
N_SB = NBLK // SB_BLOCKS        # 7
N_SUBT = 4
SUBT_ROWS = 25000               # rows per gather subtable

_kernel_cache = {}


def _build_and_compile(c_bt_key, nchunks_sbt, chunk_plan):
    """Build + compile the SPMD Bass kernel for a given static chunk layout.

    nchunks_sbt: [N_SB][N_SUBT] -> number of 128-edge chunks in that
        gather instruction.
    chunk_plan: [NBLK] -> list of (t, slot_in_sbt_tile, global_chunk_id)
        in processing order for that block.
    """
    import concourse.bacc as bacc
    import concourse.tile as tile
    import concourse.mybir as mybir
    from concourse.masks import make_identity

    NC_TOT = sum(sum(row) for row in nchunks_sbt)

    nc = bacc.Bacc(
        "TRN2",
        target_bir_lowering=False,
        debug=False,
        num_devices=N_CORES,
        num_swdge_queues=4,
    )
    f32 = mybir.dt.float32
    bf16 = mybir.dt.bfloat16
    i16 = mybir.dt.int16

    xbf = nc.dram_tensor("xbf", [NUM_NODES, D], bf16, kind="ExternalInput")
    x_shard = nc.dram_tensor("x_shard", [NODES_PER_CORE, D], f32, kind="ExternalInput")
    w_msg = nc.dram_tensor("w_msg", [D, D], f32, kind="ExternalInput")
    w_self = nc.dram_tensor("w_self", [D, D], f32, kind="ExternalInput")
    rel_bias = nc.dram_tensor("rel_bias", [NUM_REL, D], f32, kind="ExternalInput")
    bvec = nc.dram_tensor("bvec", [1, D], f32, kind="ExternalInput")
    # gather indices, already 16-partition-wrapped + replicated to 128
    n_idx_cols = sum(n * 128 // 16 for row in nchunks_sbt for n in row)
    gidx = nc.dram_tensor("gidx", [128, n_idx_cols], i16, kind="ExternalInput")
    ohw_meta = nc.dram_tensor("ohw_meta", [128, NC_TOT * 128], bf16, kind="ExternalInput")
    ohe_meta = nc.dram_tensor("ohe_meta", [128, NC_TOT * NUM_REL], bf16, kind="ExternalInput")
    out_d = nc.dram_tensor("out", [NODES_PER_CORE, D], f32, kind="ExternalOutput")

    with tile.TileContext(nc) as tc:
        with tc.tile_pool(name="const", bufs=1) as cpool, tc.tile_pool(
            name="meta", bufs=1
        ) as mpool, tc.tile_pool(name="gath", bufs=3) as gpool, tc.tile_pool(
            name="oh", bufs=6
        ) as ohpool, tc.tile_pool(name="blk", bufs=3) as bpool, tc.tile_pool(
            name="ps", bufs=2, space="PSUM"
        ) as pspool, tc.tile_pool(name="pso", bufs=2, space="PSUM") as psopool:
            # ---- constants ----
            ident = cpool.tile([128, 128], f32)
            make_identity(nc, ident[:])
            wmsg_f = cpool.tile([128, D], f32)
            nc.sync.dma_start(out=wmsg_f[:], in_=w_msg.ap())
            wmsg_b = cpool.tile([128, D], bf16)
            nc.vector.tensor_copy(out=wmsg_b[:], in_=wmsg_f[:])
            wself_f = cpool.tile([128, D], f32)
            nc.sync.dma_start(out=wself_f[:], in_=w_self.ap())
            rb_f = cpool.tile([NUM_REL, D], f32)
            nc.sync.dma_start(out=rb_f[:], in_=rel_bias.ap())
            rb_b = cpool.tile([NUM_REL, D], bf16)
            nc.vector.tensor_copy(out=rb_b[:], in_=rb_f[:])
            b_row = cpool.tile([1, D], f32)
            nc.sync.dma_start(out=b_row[:], in_=bvec.ap())
            ones1 = cpool.tile([1, D], f32)
            nc.vector.memset(ones1[:], 1.0)

            # ---- gather indices (one DMA) ----
            gidx_t = mpool.tile([128, n_idx_cols], i16)
            nc.sync.dma_start(out=gidx_t[:], in_=gidx.ap())

            # precompute static offsets
            idx_off = {}
            off = 0
            for sb in range(N_SB):
                for t in range(N_SUBT):
                    idx_off[(sb, t)] = off
                    off += nchunks_sbt[sb][t] * 128 // 16

            gmax = [max(nchunks_sbt[sb][t] for sb in range(N_SB)) for t in range(N_SUBT)]
            pos_of = {}
            _p = 0
            for _b in range(NBLK):
                pos_of[_b] = _p
                _p += len(chunk_plan[_b])

            PIECE = 16  # chunks per gather instruction (2048 idxs)
            swdge_i = 0
            for sb in range(N_SB):
                # ---- gather instructions for this superblock, in pieces ----
                gtiles = []
                for t in range(N_SUBT):
                    nck = nchunks_sbt[sb][t]
                    gt = gpool.tile([128, gmax[t] * 128], bf16, tag=f"g{t}")
                    base = t * SUBT_ROWS
                    rows = min(SUBT_ROWS, NUM_NODES - base)
                    io = idx_off[(sb, t)]
                    for p0 in range(0, nck, PIECE):
                        pk = min(PIECE, nck - p0)
                        n = pk * 128
                        nc.gpsimd.dma_gather(
                            out_ap=gt[:, p0 * 128 : (p0 + pk) * 128].rearrange(
                                "p (c r) -> p c r", r=128
                            ),
                            in_ap=xbf.ap()[base : base + rows, :],
                            idxs_ap=gidx_t[
                                :, io + p0 * 8 : io + (p0 + pk) * 8
                            ],
                            num_idxs=n,
                            num_idxs_reg=n,
                            elem_size=D,
                            single_packet=False,
                            queue_num=swdge_i % 4,
                        )
                        swdge_i += 1
                    gtiles.append(gt)

                for bi in range(SB_BLOCKS):
                    blk = sb * SB_BLOCKS + bi
                    plan = chunk_plan[blk]
                    assert plan, f"block {blk} has no chunks"
                    nchunk = len(plan)
                    pos0 = pos_of[blk]
                    ohw_b = ohpool.tile([128, nchunk * 128], bf16, tag="ohw")
                    nc.sync.dma_start(
                        out=ohw_b[:],
                        in_=ohw_meta.ap()[:, pos0 * 128 : (pos0 + nchunk) * 128],
                    )
                    ohe_b = ohpool.tile([128, nchunk * NUM_REL], bf16, tag="ohe")
                    nc.sync.dma_start(
                        out=ohe_b[:],
                        in_=ohe_meta.ap()[
                            :, pos0 * NUM_REL : (pos0 + nchunk) * NUM_REL
                        ],
                    )
                    sT = pspool.tile([128, 128], f32, tag="sT")
                    cT = pspool.tile([NUM_REL, 128], f32, tag="cT")
                    for ci, (t, slot, gchunk) in enumerate(plan):
                        ohw = ohw_b[:, ci * 128 : (ci + 1) * 128]
                        ohe = ohe_b[:, ci * NUM_REL : (ci + 1) * NUM_REL]
                        xg = gtiles[t][:, slot * 128 : (slot + 1) * 128]
                        nc.tensor.matmul(
                            out=sT[:], lhsT=xg, rhs=ohw,
                            start=(ci == 0), stop=(ci == nchunk - 1),
                        )
                        nc.tensor.matmul(
                            out=cT[:], lhsT=ohe, rhs=ohw,
                            start=(ci == 0), stop=(ci == nchunk - 1),
                        )

                    # ---- block epilogue ----
                    sT_sb = bpool.tile([128, 128], bf16, tag="sTsb")
                    nc.vector.tensor_copy(out=sT_sb[:], in_=sT[:])
                    cT_sb = bpool.tile([NUM_REL, 128], bf16, tag="cTsb")
                    nc.vector.tensor_copy(out=cT_sb[:], in_=cT[:])
                    x_blk = bpool.tile([128, D], f32, tag="xblk")
                    nc.sync.dma_start(
                        out=x_blk[:], in_=x_shard.ap()[blk * 128 : (blk + 1) * 128, :]
                    )
                    xT_ps = psopool.tile([128, 128], f32, tag="xT")
                    nc.tensor.transpose(out=xT_ps[:], in_=x_blk[:], identity=ident[:])
                    xT_sb = bpool.tile([128, 128], f32, tag="xTsb")
                    nc.vector.tensor_copy(out=xT_sb[:], in_=xT_ps[:])

                    acc = psopool.tile([128, D], f32, tag="acc")
                    nc.tensor.matmul(
                        out=acc[:], lhsT=sT_sb[:], rhs=wmsg_b[:],
                        start=True, stop=False,
                    )
                    nc.tensor.matmul(
                        out=acc[:], lhsT=cT_sb[:], rhs=rb_b[:],
                        start=False, stop=False,
                    )
                    nc.tensor.matmul(
                        out=acc[:], lhsT=xT_sb[:], rhs=wself_f[:],
                        start=False, stop=False,
                    )
                    nc.tensor.matmul(
                        out=acc[:], lhsT=ones1[:], rhs=b_row[:],
                        start=False, stop=True,
                    )
                    o_sb = bpool.tile([128, D], f32, tag="osb")
                    nc.scalar.activation(
                        out=o_sb[:], in_=acc[:], func=mybir.ActivationFunctionType.Relu
                    )
                    nc.sync.dma_start(
                        out=out_d.ap()[blk * 128 : (blk + 1) * 128, :], in_=o_sb[:]
                    )

    nc.compile()
    return nc


def _prep(inputs):
    """Host-side sharding/layout. Returns (in_maps, static_key, layout)."""
    x = np.ascontiguousarray(np.asarray(inputs["x"], dtype=np.float32))
    source = np.asarray(inputs["source"]).astype(np.int64)
    target = np.asarray(inputs["target"]).astype(np.int64)
    edge_type = np.asarray(inputs["edge_type"]).astype(np.int64)
    ew = np.asarray(inputs["edge_weights"], dtype=np.float32)
    w_msg = np.ascontiguousarray(np.asarray(inputs["W_msg"], dtype=np.float32))
    rel_bias = np.ascontiguousarray(np.asarray(inputs["rel_bias"], dtype=np.float32))
    w_self = np.ascontiguousarray(np.asarray(inputs["W_self"], dtype=np.float32))
    b = np.asarray(inputs["b"], dtype=np.float32).reshape(1, D)

    n = x.shape[0]
    assert n == NUM_NODES

    xbf = x.astype(ml_dtypes.bfloat16)

    core = target // NODES_PER_CORE
    tgt_local = target - core * NODES_PER_CORE
    blk = tgt_local >> 7
    tgt_in_blk = tgt_local & 127
    subt = source // SUBT_ROWS
    src_local = source - subt * SUBT_ROWS

    # per (core, blk, subtable) edge index lists
    # order edges by (core, blk, subt) with a stable sort
    key = ((core * NBLK + blk) * N_SUBT + subt).astype(np.int64)
    order = np.argsort(key, kind="stable")
    key_s = key[order]
    # group boundaries
    uniq, starts = np.unique(key_s, return_index=True)
    counts = np.diff(np.append(starts, key_s.shape[0]))

    cnt = np.zeros((N_CORES, NBLK, N_SUBT), dtype=np.int64)
    ci = uniq // (NBLK * N_SUBT)
    bi = (uniq // N_SUBT) % NBLK
    ti = uniq % N_SUBT
    cnt[ci, bi, ti] = counts

    # static chunk capacity per (blk, subtable): max over cores
    c_bt = np.ceil(cnt.max(axis=0) / 128).astype(np.int64)  # (NBLK, N_SUBT)
    # ensure every block has at least one chunk
    empty = c_bt.sum(axis=1) == 0
    c_bt[empty, 0] = 1

    nchunks_sbt = [
        [int(c_bt[sb * SB_BLOCKS : (sb + 1) * SB_BLOCKS, t].sum()) for t in range(N_SUBT)]
        for sb in range(N_SB)
    ]
    NC_TOT = int(c_bt.sum())

    # global chunk ids: order is (sb, t, blk-within-sb, chunk)
    gchunk_of = np.zeros((NBLK, N_SUBT), dtype=np.int64)  # first chunk id
    slot_of = np.zeros((NBLK, N_SUBT), dtype=np.int64)    # first slot in (sb,t) tile
    g = 0
    for sb in range(N_SB):
        for t in range(N_SUBT):
            s = 0
            for bi2 in range(SB_BLOCKS):
                bb = sb * SB_BLOCKS + bi2
                gchunk_of[bb, t] = g
                slot_of[bb, t] = s
                g += int(c_bt[bb, t])
                s += int(c_bt[bb, t])
    assert g == NC_TOT

    chunk_plan = []
    for bb in range(NBLK):
        plan = []
        for t in range(N_SUBT):
            for c in range(int(c_bt[bb, t])):
                plan.append((t, int(slot_of[bb, t] + c), int(gchunk_of[bb, t] + c)))
        chunk_plan.append(plan)

    # position of each block's chunk run in the (block-major) onehot layout
    pos_of_blk = np.zeros(NBLK, dtype=np.int64)
    p = 0
    for bb in range(NBLK):
        pos_of_blk[bb] = p
        p += len(chunk_plan[bb])
    # gchunk -> block-major position
    pos_of_gchunk = np.zeros(NC_TOT, dtype=np.int64)
    for bb in range(NBLK):
        for i, (_t, _s, g2) in enumerate(chunk_plan[bb]):
            pos_of_gchunk[g2] = pos_of_blk[bb] + i

    n_idx_cols = sum(nc_ * 128 // 16 for row in nchunks_sbt for nc_ in row)

    # build per-core tensors
    in_maps = []
    # offsets of edge groups in the sorted edge array, per core
    start_of = {}
    for u, s0, c0 in zip(uniq, starts, counts):
        start_of[int(u)] = (int(s0), int(c0))

    for c in range(N_CORES):
        gidx = np.zeros((128, n_idx_cols), dtype=np.int16)
        ohw_m = np.zeros((128, NC_TOT * 128), dtype=ml_dtypes.bfloat16)
        ohe_m = np.zeros((128, NC_TOT * NUM_REL), dtype=ml_dtypes.bfloat16)

        icol = 0
        for sb in range(N_SB):
            for t in range(N_SUBT):
                nck = nchunks_sbt[sb][t]
                if nck == 0:
                    continue
                nslots = nck * 128
                idxs = np.zeros(nslots, dtype=np.int16)
                for bi2 in range(SB_BLOCKS):
                    bb = sb * SB_BLOCKS + bi2
                    k = (c * NBLK + bb) * N_SUBT + t
                    s0, n_e = start_of.get(k, (0, 0))
                    sl0 = int(slot_of[bb, t]) * 128 - int(slot_of[sb * SB_BLOCKS, t]) * 128
                    g0 = int(gchunk_of[bb, t])
                    if n_e:
                        eids = order[s0 : s0 + n_e]
                        idxs[sl0 : sl0 + n_e] = src_local[eids].astype(np.int16)
                        # meta: chunk-major [128 partitions]
                        for cc in range(int(c_bt[bb, t])):
                            lo = cc * 128
                            hi = min(n_e, lo + 128)
                            if hi <= lo:
                                break
                            ecol = eids[lo:hi]
                            gc = g0 + cc
                            npart = hi - lo
                            pos = int(pos_of_gchunk[gc])
                            parts = np.arange(npart)
                            ohw_m[parts, pos * 128 + tgt_in_blk[ecol]] = ew[
                                ecol
                            ].astype(ml_dtypes.bfloat16)
                            ohe_m[parts, pos * NUM_REL + edge_type[ecol]] = 1.0
                # wrap idxs: element j -> partition j%16, col j//16; replicate x8
                wrapped = idxs.reshape(nslots // 16, 16).T  # (16, nslots/16)
                gidx[:, icol : icol + nslots // 16] = np.tile(wrapped, (8, 1))
                icol += nslots // 16
        assert icol == n_idx_cols

        xs = np.zeros((NODES_PER_CORE, D), dtype=np.float32)
        lo = c * NODES_PER_CORE
        hi = min(lo + NODES_PER_CORE, NUM_NODES)
        xs[: hi - lo] = x[lo:hi]

        in_maps.append(
            {
                "xbf": xbf,
                "x_shard": xs,
                "w_msg": w_msg,
                "w_self": w_self,
                "rel_bias": rel_bias,
                "bvec": b,
                "gidx": gidx,
                "ohw_meta": ohw_m,
                "ohe_meta": ohe_m,
            }
        )

    static_key = tuple(c_bt.flatten().tolist())
    return in_maps, static_key, (nchunks_sbt, chunk_plan)


def kernel(**inputs) -> np.ndarray:
    from concourse import bass_utils

    in_maps, static_key, (nchunks_sbt, chunk_plan) = _prep(inputs)

    nc = _kernel_cache.get(static_key)
    if nc is None:
        nc = _build_and_compile(static_key, nchunks_sbt, chunk_plan)
        _kernel_cache[static_key] = nc

    res = bass_utils.run_bass_kernel_spmd(
        nc, in_maps, core_ids=list(range(N_CORES))
    )
    parts = [res.results[c]["out"] for c in range(N_CORES)]
    full = np.concatenate(parts, axis=0)[:NUM_NODES]
    return full.astype(np.float32)


# revision 9
# speedup vs baseline: 1.4057x; 1.0263x over previous
"""Trainium2 Bass kernel for nn_MessagePassingBlock (GNN message passing).

Math (reference):
    h     = x @ W_msg                       # (N, D)
    msg   = (h[source] + rel_bias[edge_type]) * edge_weights[:, None]
    delta = segment_sum(msg, target, N)     # (N, D)
    out   = relu(x @ W_self + delta + b)

Distribution: target-sharded across 8 cores (no collectives). Core c owns
nodes [c*12544, (c+1)*12544); every edge lives on its target's core.

Per-core algorithm (all matmul-based, no per-edge transposes):
  For each 128-node target block b, accumulate over that block's edges
  (chunks of 128 edges, gathered via batched SWDGE dma_gather from a bf16
  mirror of x):
      sT[k, j] += sum_e xg[e, k] * w_e * [tgt_e == j]      (PE, bf16)
      CT[r, j] += sum_e [et_e == r] * w_e * [tgt_e == j]   (PE, bf16)
  then
      out_b = relu(sT^T @ W_msg + CT^T @ rel_bias + x_b @ W_self + b)
  The onehot operands are built with single fused DVE tensor_scalar ops.
  Edge weights are folded into the target-onehot; padding edges carry w=0
  so they contribute exactly zero (self-masking).

Gather: x is split into 4 row subtables (<=32767 rows, int16 indices);
one dma_gather instruction per (superblock of 14 blocks, subtable), spread
across the 4 SWDGE queues.
"""

import functools
import math

import numpy as np
import ml_dtypes

NUM_NODES = 100000
D = 128
NUM_REL = 8
N_CORES = 8
NODES_PER_CORE = 12544          # 98 blocks of 128
NBLK = NODES_PER_CORE // 128    # 98
SB_BLOCKS = 14                  # blocks per superblock
N_SB = NBLK // SB_BLOCKS        # 7
N_SUBT = 4
SUBT_ROWS = 25000               # rows per gather subtable

_kernel_cache = {}


def _build_and_compile(c_bt_key, nchunks_sbt, chunk_plan):
    """Build + compile the SPMD Bass kernel for a given static chunk layout.

    nchunks_sbt: [N_SB][N_SUBT] -> number of 128-edge chunks in that
        gather instruction.
    chunk_plan: [NBLK] -> list of (t, slot_in_sbt_tile, global_chunk_id)
        in processing order for that block.
    """
    import concourse.bacc as bacc
    import concourse.tile as tile
    import concourse.mybir as mybir
    from concourse.masks import make_identity

    NC_TOT = sum(sum(row) for row in nchunks_sbt)

    nc = bacc.Bacc(
        "TRN2",
        target_bir_lowering=False,
        debug=False,
        num_devices=N_CORES,
        num_swdge_queues=4,
    )
    f32 = mybir.dt.float32
    bf16 = mybir.dt.bfloat16
    i16 = mybir.dt.int16

    xbf = nc.dram_tensor("xbf", [NUM_NODES, D], bf16, kind="ExternalInput")
    x_shard = nc.dram_tensor("x_shard", [NODES_PER_CORE, D], f32, kind="ExternalInput")
    w_msg = nc.dram_tensor("w_msg", [D, D], f32, kind="ExternalInput")
    w_self = nc.dram_tensor("w_self", [D, D], f32, kind="ExternalInput")
    rel_bias = nc.dram_tensor("rel_bias", [NUM_REL, D], f32, kind="ExternalInput")
    bvec = nc.dram_tensor("bvec", [1, D], f32, kind="ExternalInput")
    # gather indices, already 16-partition-wrapped + replicated to 128
    n_idx_cols = sum(n * 128 // 16 for row in nchunks_sbt for n in row)
    gidx = nc.dram_tensor("gidx", [128, n_idx_cols], i16, kind="ExternalInput")
    ohw_meta = nc.dram_tensor("ohw_meta", [128, NC_TOT * 128], bf16, kind="ExternalInput")
    ohe_meta = nc.dram_tensor("ohe_meta", [128, NC_TOT * NUM_REL], bf16, kind="ExternalInput")
    out_d = nc.dram_tensor("out", [NODES_PER_CORE, D], f32, kind="ExternalOutput")

    with tile.TileContext(nc) as tc:
        with tc.tile_pool(name="const", bufs=1) as cpool, tc.tile_pool(
            name="meta", bufs=1
        ) as mpool, tc.tile_pool(name="gath", bufs=2) as gpool, tc.tile_pool(
            name="oh", bufs=2
        ) as ohpool, tc.tile_pool(name="blk", bufs=3) as bpool, tc.tile_pool(
            name="ps", bufs=2, space="PSUM"
        ) as pspool, tc.tile_pool(name="pso", bufs=2, space="PSUM") as psopool:
            # ---- constants ----
            ident = cpool.tile([128, 128], f32)
            make_identity(nc, ident[:])
            wmsg_f = cpool.tile([128, D], f32)
            nc.sync.dma_start(out=wmsg_f[:], in_=w_msg.ap())
            wmsg_b = cpool.tile([128, D], bf16)
            nc.vector.tensor_copy(out=wmsg_b[:], in_=wmsg_f[:])
            wself_f = cpool.tile([128, D], f32)
            nc.sync.dma_start(out=wself_f[:], in_=w_self.ap())
            rb_f = cpool.tile([NUM_REL, D], f32)
            nc.sync.dma_start(out=rb_f[:], in_=rel_bias.ap())
            rb_b = cpool.tile([NUM_REL, D], bf16)
            nc.vector.tensor_copy(out=rb_b[:], in_=rb_f[:])
            b_row = cpool.tile([1, D], f32)
            nc.sync.dma_start(out=b_row[:], in_=bvec.ap())
            ones1 = cpool.tile([1, D], f32)
            nc.vector.memset(ones1[:], 1.0)

            # ---- gather indices (one DMA) ----
            gidx_t = mpool.tile([128, n_idx_cols], i16)
            nc.sync.dma_start(out=gidx_t[:], in_=gidx.ap())

            # precompute static offsets
            idx_off = {}
            off = 0
            for sb in range(N_SB):
                for t in range(N_SUBT):
                    idx_off[(sb, t)] = off
                    off += nchunks_sbt[sb][t] * 128 // 16

            gmax = [max(nchunks_sbt[sb][t] for sb in range(N_SB)) for t in range(N_SUBT)]
            _starts = []
            for _g in range(0, NBLK, 7):
                _e = _g + 7
                _p0 = 0
                for _b in range(_g):
                    _p0 += len(chunk_plan[_b])
                _p1 = _p0
                for _b in range(_g, min(_e, NBLK)):
                    _p1 += len(chunk_plan[_b])
                _starts.append(_p1 - _p0)
            ghw_max = max(_starts)
            pos_of = {}
            _p = 0
            for _b in range(NBLK):
                pos_of[_b] = _p
                _p += len(chunk_plan[_b])

            PIECE = 16  # chunks per gather instruction (2048 idxs)
            swdge_i = 0
            for sb in range(N_SB):
                # ---- gather instructions for this superblock, in pieces ----
                gtiles = []
                for t in range(N_SUBT):
                    nck = nchunks_sbt[sb][t]
                    gt = gpool.tile([128, gmax[t] * 128], bf16, tag=f"g{t}")
                    base = t * SUBT_ROWS
                    rows = min(SUBT_ROWS, NUM_NODES - base)
                    io = idx_off[(sb, t)]
                    for p0 in range(0, nck, PIECE):
                        pk = min(PIECE, nck - p0)
                        n = pk * 128
                        nc.gpsimd.dma_gather(
                            out_ap=gt[:, p0 * 128 : (p0 + pk) * 128].rearrange(
                                "p (c r) -> p c r", r=128
                            ),
                            in_ap=xbf.ap()[base : base + rows, :],
                            idxs_ap=gidx_t[
                                :, io + p0 * 8 : io + (p0 + pk) * 8
                            ],
                            num_idxs=n,
                            num_idxs_reg=n,
                            elem_size=D,
                            single_packet=False,
                            queue_num=swdge_i % 4,
                        )
                        swdge_i += 1
                    gtiles.append(gt)

                for half in range(2):
                    g0 = sb * SB_BLOCKS + half * 7
                    p0 = pos_of[g0]
                    p1 = pos_of[g0 + 7] if g0 + 7 < NBLK else NC_TOT
                    nchv = p1 - p0
                    ghw = ohpool.tile([128, ghw_max * 128], bf16, tag="ghw")
                    nc.scalar.dma_start(
                        out=ghw[:, : nchv * 128],
                        in_=ohw_meta.ap()[:, p0 * 128 : p1 * 128],
                    )
                    ghe = ohpool.tile([128, ghw_max * NUM_REL], bf16, tag="ghe")
                    nc.scalar.dma_start(
                        out=ghe[:, : nchv * NUM_REL],
                        in_=ohe_meta.ap()[:, p0 * NUM_REL : p1 * NUM_REL],
                    )
                    x7 = bpool.tile([128, 7 * 128], f32, tag="x7")
                    nc.sync.dma_start(
                        out=x7[:],
                        in_=x_shard.ap()[g0 * 128 : (g0 + 7) * 128, :].rearrange(
                            "(c p) f -> p c f", p=128
                        ),
                    )
                    o7 = bpool.tile([128, 7 * 128], f32, tag="o7")
                    for bi in range(7):
                        blk = g0 + bi
                        plan = chunk_plan[blk]
                        assert plan, f"block {blk} has no chunks"
                        nchunk = len(plan)
                        bpos = pos_of[blk] - p0
                        sT = pspool.tile([128, 128], f32, tag="sT")
                        cT = pspool.tile([NUM_REL, 128], f32, tag="cT")
                        for ci, (t, slot, gchunk) in enumerate(plan):
                            ohw = ghw[:, (bpos + ci) * 128 : (bpos + ci + 1) * 128]
                            ohe = ghe[
                                :,
                                (bpos + ci) * NUM_REL : (bpos + ci + 1) * NUM_REL,
                            ]
                            xg = gtiles[t][:, slot * 128 : (slot + 1) * 128]
                            nc.tensor.matmul(
                                out=sT[:], lhsT=xg, rhs=ohw,
                                start=(ci == 0), stop=(ci == nchunk - 1),
                            )
                            nc.tensor.matmul(
                                out=cT[:], lhsT=ohe, rhs=ohw,
                                start=(ci == 0), stop=(ci == nchunk - 1),
                            )

                        # ---- block epilogue ----
                        sT_sb = bpool.tile([128, 128], bf16, tag="sTsb")
                        nc.vector.tensor_copy(out=sT_sb[:], in_=sT[:])
                        cT_sb = bpool.tile([NUM_REL, 128], bf16, tag="cTsb")
                        nc.vector.tensor_copy(out=cT_sb[:], in_=cT[:])
                        x_blk = x7[:, bi * 128 : (bi + 1) * 128]
                        xT_ps = psopool.tile([128, 128], f32, tag="xT")
                        nc.tensor.transpose(
                            out=xT_ps[:], in_=x_blk, identity=ident[:]
                        )
                        xT_sb = bpool.tile([128, 128], f32, tag="xTsb")
                        nc.vector.tensor_copy(out=xT_sb[:], in_=xT_ps[:])

                        acc = psopool.tile([128, D], f32, tag="acc")
                        nc.tensor.matmul(
                            out=acc[:], lhsT=sT_sb[:], rhs=wmsg_b[:],
                            start=True, stop=False,
                        )
                        nc.tensor.matmul(
                            out=acc[:], lhsT=cT_sb[:], rhs=rb_b[:],
                            start=False, stop=False,
                        )
                        nc.tensor.matmul(
                            out=acc[:], lhsT=xT_sb[:], rhs=wself_f[:],
                            start=False, stop=False,
                        )
                        nc.tensor.matmul(
                            out=acc[:], lhsT=ones1[:], rhs=b_row[:],
                            start=False, stop=True,
                        )
                        nc.scalar.activation(
                            out=o7[:, bi * 128 : (bi + 1) * 128],
                            in_=acc[:],
                            func=mybir.ActivationFunctionType.Relu,
                        )
                    nc.sync.dma_start(
                        out=out_d.ap()[g0 * 128 : (g0 + 7) * 128, :].rearrange(
                            "(c p) f -> p c f", p=128
                        ),
                        in_=o7[:],
                    )

    nc.compile()
    return nc


def _prep(inputs):
    """Host-side sharding/layout. Returns (in_maps, static_key, layout)."""
    x = np.ascontiguousarray(np.asarray(inputs["x"], dtype=np.float32))
    source = np.asarray(inputs["source"]).astype(np.int64)
    target = np.asarray(inputs["target"]).astype(np.int64)
    edge_type = np.asarray(inputs["edge_type"]).astype(np.int64)
    ew = np.asarray(inputs["edge_weights"], dtype=np.float32)
    w_msg = np.ascontiguousarray(np.asarray(inputs["W_msg"], dtype=np.float32))
    rel_bias = np.ascontiguousarray(np.asarray(inputs["rel_bias"], dtype=np.float32))
    w_self = np.ascontiguousarray(np.asarray(inputs["W_self"], dtype=np.float32))
    b = np.asarray(inputs["b"], dtype=np.float32).reshape(1, D)

    n = x.shape[0]
    assert n == NUM_NODES

    xbf = x.astype(ml_dtypes.bfloat16)

    core = target // NODES_PER_CORE
    tgt_local = target - core * NODES_PER_CORE
    blk = tgt_local >> 7
    tgt_in_blk = tgt_local & 127
    subt = source // SUBT_ROWS
    src_local = source - subt * SUBT_ROWS

    # per (core, blk, subtable) edge index lists
    # order edges by (core, blk, subt) with a stable sort
    key = ((core * NBLK + blk) * N_SUBT + subt).astype(np.int64)
    order = np.argsort(key, kind="stable")
    key_s = key[order]
    # group boundaries
    uniq, starts = np.unique(key_s, return_index=True)
    counts = np.diff(np.append(starts, key_s.shape[0]))

    cnt = np.zeros((N_CORES, NBLK, N_SUBT), dtype=np.int64)
    ci = uniq // (NBLK * N_SUBT)
    bi = (uniq // N_SUBT) % NBLK
    ti = uniq % N_SUBT
    cnt[ci, bi, ti] = counts

    # static chunk capacity per (blk, subtable): max over cores
    c_bt = np.ceil(cnt.max(axis=0) / 128).astype(np.int64)  # (NBLK, N_SUBT)
    # ensure every block has at least one chunk
    empty = c_bt.sum(axis=1) == 0
    c_bt[empty, 0] = 1

    nchunks_sbt = [
        [int(c_bt[sb * SB_BLOCKS : (sb + 1) * SB_BLOCKS, t].sum()) for t in range(N_SUBT)]
        for sb in range(N_SB)
    ]
    NC_TOT = int(c_bt.sum())

    # global chunk ids: order is (sb, t, blk-within-sb, chunk)
    gchunk_of = np.zeros((NBLK, N_SUBT), dtype=np.int64)  # first chunk id
    slot_of = np.zeros((NBLK, N_SUBT), dtype=np.int64)    # first slot in (sb,t) tile
    g = 0
    for sb in range(N_SB):
        for t in range(N_SUBT):
            s = 0
            for bi2 in range(SB_BLOCKS):
                bb = sb * SB_BLOCKS + bi2
                gchunk_of[bb, t] = g
                slot_of[bb, t] = s
                g += int(c_bt[bb, t])
                s += int(c_bt[bb, t])
    assert g == NC_TOT

    chunk_plan = []
    for bb in range(NBLK):
        plan = []
        for t in range(N_SUBT):
            for c in range(int(c_bt[bb, t])):
                plan.append((t, int(slot_of[bb, t] + c), int(gchunk_of[bb, t] + c)))
        chunk_plan.append(plan)

    # position of each block's chunk run in the (block-major) onehot layout
    pos_of_blk = np.zeros(NBLK, dtype=np.int64)
    p = 0
    for bb in range(NBLK):
        pos_of_blk[bb] = p
        p += len(chunk_plan[bb])
    # gchunk -> block-major position
    pos_of_gchunk = np.zeros(NC_TOT, dtype=np.int64)
    for bb in range(NBLK):
        for i, (_t, _s, g2) in enumerate(chunk_plan[bb]):
            pos_of_gchunk[g2] = pos_of_blk[bb] + i

    n_idx_cols = sum(nc_ * 128 // 16 for row in nchunks_sbt for nc_ in row)

    # build per-core tensors
    in_maps = []
    # offsets of edge groups in the sorted edge array, per core
    start_of = {}
    for u, s0, c0 in zip(uniq, starts, counts):
        start_of[int(u)] = (int(s0), int(c0))

    for c in range(N_CORES):
        gidx = np.zeros((128, n_idx_cols), dtype=np.int16)
        ohw_m = np.zeros((128, NC_TOT * 128), dtype=ml_dtypes.bfloat16)
        ohe_m = np.zeros((128, NC_TOT * NUM_REL), dtype=ml_dtypes.bfloat16)

        icol = 0
        for sb in range(N_SB):
            for t in range(N_SUBT):
                nck = nchunks_sbt[sb][t]
                if nck == 0:
                    continue
                nslots = nck * 128
                idxs = np.zeros(nslots, dtype=np.int16)
                for bi2 in range(SB_BLOCKS):
                    bb = sb * SB_BLOCKS + bi2
                    k = (c * NBLK + bb) * N_SUBT + t
                    s0, n_e = start_of.get(k, (0, 0))
                    sl0 = int(slot_of[bb, t]) * 128 - int(slot_of[sb * SB_BLOCKS, t]) * 128
                    g0 = int(gchunk_of[bb, t])
                    if n_e:
                        eids = order[s0 : s0 + n_e]
                        idxs[sl0 : sl0 + n_e] = src_local[eids].astype(np.int16)
                        # meta: chunk-major [128 partitions]
                        for cc in range(int(c_bt[bb, t])):
                            lo = cc * 128
                            hi = min(n_e, lo + 128)
                            if hi <= lo:
                                break
                            ecol = eids[lo:hi]
                            gc = g0 + cc
                            npart = hi - lo
                            pos = int(pos_of_gchunk[gc])
                            parts = np.arange(npart)
                            ohw_m[parts, pos * 128 + tgt_in_blk[ecol]] = ew[
                                ecol
                            ].astype(ml_dtypes.bfloat16)
                            ohe_m[parts, pos * NUM_REL + edge_type[ecol]] = 1.0
                # wrap idxs: element j -> partition j%16, col j//16; replicate x8
                wrapped = idxs.reshape(nslots // 16, 16).T  # (16, nslots/16)
                gidx[:, icol : icol + nslots // 16] = np.tile(wrapped, (8, 1))
                icol += nslots // 16
        assert icol == n_idx_cols

        xs = np.zeros((NODES_PER_CORE, D), dtype=np.float32)
        lo = c * NODES_PER_CORE
        hi = min(lo + NODES_PER_CORE, NUM_NODES)
        xs[: hi - lo] = x[lo:hi]

        in_maps.append(
            {
                "xbf": xbf,
                "x_shard": xs,
                "w_msg": w_msg,
                "w_self": w_self,
                "rel_bias": rel_bias,
                "bvec": b,
                "gidx": gidx,
                "ohw_meta": ohw_m,
                "ohe_meta": ohe_m,
            }
        )

    static_key = tuple(c_bt.flatten().tolist())
    return in_maps, static_key, (nchunks_sbt, chunk_plan)


def kernel(**inputs) -> np.ndarray:
    from concourse import bass_utils

    in_maps, static_key, (nchunks_sbt, chunk_plan) = _prep(inputs)

    nc = _kernel_cache.get(static_key)
    if nc is None:
        nc = _build_and_compile(static_key, nchunks_sbt, chunk_plan)
        _kernel_cache[static_key] = nc

    res = bass_utils.run_bass_kernel_spmd(
        nc, in_maps, core_ids=list(range(N_CORES))
    )
    parts = [res.results[c]["out"] for c in range(N_CORES)]
    full = np.concatenate(parts, axis=0)[:NUM_NODES]
    return full.astype(np.float32)


# revision 10
# speedup vs baseline: 1.8432x; 1.3112x over previous
"""Trainium2 Bass kernel for nn_MessagePassingBlock (GNN message passing).

Math (reference):
    h     = x @ W_msg                       # (N, D)
    msg   = (h[source] + rel_bias[edge_type]) * edge_weights[:, None]
    delta = segment_sum(msg, target, N)     # (N, D)
    out   = relu(x @ W_self + delta + b)

Distribution: target-sharded across 8 cores (no collectives). Core c owns
nodes [c*12544, (c+1)*12544); every edge lives on its target's core.

Per-core algorithm (all matmul-based, no per-edge transposes):
  For each 128-node target block b, accumulate over that block's edges
  (chunks of 128 edges, gathered via batched SWDGE dma_gather from a bf16
  mirror of x):
      sT[k, j] += sum_e xg[e, k] * w_e * [tgt_e == j]      (PE, bf16)
      CT[r, j] += sum_e [et_e == r] * w_e * [tgt_e == j]   (PE, bf16)
  then
      out_b = relu(sT^T @ W_msg + CT^T @ rel_bias + x_b @ W_self + b)
  The onehot operands are built with single fused DVE tensor_scalar ops.
  Edge weights are folded into the target-onehot; padding edges carry w=0
  so they contribute exactly zero (self-masking).

Gather: x is split into 4 row subtables (<=32767 rows, int16 indices);
one dma_gather instruction per (superblock of 14 blocks, subtable), spread
across the 4 SWDGE queues.
"""

import functools
import math

import numpy as np
import ml_dtypes

NUM_NODES = 100000
D = 128
NUM_REL = 8
N_CORES = 8
NODES_PER_CORE = 12544          # 98 blocks of 128
NBLK = NODES_PER_CORE // 128    # 98
SB_BLOCKS = 14                  # blocks per superblock
N_SB = NBLK // SB_BLOCKS        # 7
N_SUBT = 4
SUBT_ROWS = 25000               # rows per gather subtable

_kernel_cache = {}


def _build_and_compile(c_bt_key, nchunks_sbt, chunk_plan):
    """Build + compile the SPMD Bass kernel for a given static chunk layout.

    nchunks_sbt: [N_SB][N_SUBT] -> number of 128-edge chunks in that
        gather instruction.
    chunk_plan: [NBLK] -> list of (t, slot_in_sbt_tile, global_chunk_id)
        in processing order for that block.
    """
    import concourse.bacc as bacc
    import concourse.tile as tile
    import concourse.mybir as mybir
    from concourse.masks import make_identity

    NC_TOT = sum(sum(row) for row in nchunks_sbt)

    nc = bacc.Bacc(
        "TRN2",
        target_bir_lowering=False,
        debug=False,
        num_devices=N_CORES,
        num_swdge_queues=4,
    )
    f32 = mybir.dt.float32
    bf16 = mybir.dt.bfloat16
    i16 = mybir.dt.int16

    xbf = nc.dram_tensor("xbf", [NUM_NODES, D], bf16, kind="ExternalInput")
    x_shard = nc.dram_tensor("x_shard", [NODES_PER_CORE, D], f32, kind="ExternalInput")
    w_msg = nc.dram_tensor("w_msg", [D, D], f32, kind="ExternalInput")
    w_self = nc.dram_tensor("w_self", [D, D], f32, kind="ExternalInput")
    rel_bias = nc.dram_tensor("rel_bias", [NUM_REL, D], f32, kind="ExternalInput")
    bvec = nc.dram_tensor("bvec", [1, D], f32, kind="ExternalInput")
    # gather indices, already 16-partition-wrapped + replicated to 128
    n_idx_cols = sum(n * 128 // 16 for row in nchunks_sbt for n in row)
    gidx = nc.dram_tensor("gidx", [128, n_idx_cols], i16, kind="ExternalInput")
    ohw_meta = nc.dram_tensor("ohw_meta", [128, NC_TOT * 128], bf16, kind="ExternalInput")
    ohe_meta = nc.dram_tensor("ohe_meta", [128, NC_TOT * NUM_REL], bf16, kind="ExternalInput")
    out_d = nc.dram_tensor("out", [NODES_PER_CORE, D], f32, kind="ExternalOutput")

    with tile.TileContext(nc) as tc:
        with tc.tile_pool(name="const", bufs=1) as cpool, tc.tile_pool(
            name="meta", bufs=1
        ) as mpool, tc.tile_pool(name="gath", bufs=2) as gpool, tc.tile_pool(
            name="oh", bufs=2
        ) as ohpool, tc.tile_pool(name="blk", bufs=3) as bpool, tc.tile_pool(
            name="ps", bufs=2, space="PSUM"
        ) as pspool, tc.tile_pool(name="pso", bufs=2, space="PSUM") as psopool:
            # ---- constants ----
            ident = cpool.tile([128, 128], f32)
            make_identity(nc, ident[:])
            wmsg_f = cpool.tile([128, D], f32)
            nc.sync.dma_start(out=wmsg_f[:], in_=w_msg.ap())
            wmsg_b = cpool.tile([128, D], bf16)
            nc.vector.tensor_copy(out=wmsg_b[:], in_=wmsg_f[:])
            wself_f = cpool.tile([128, D], f32)
            nc.sync.dma_start(out=wself_f[:], in_=w_self.ap())
            rb_f = cpool.tile([NUM_REL, D], f32)
            nc.sync.dma_start(out=rb_f[:], in_=rel_bias.ap())
            rb_b = cpool.tile([NUM_REL, D], bf16)
            nc.vector.tensor_copy(out=rb_b[:], in_=rb_f[:])
            b_row = cpool.tile([1, D], f32)
            nc.sync.dma_start(out=b_row[:], in_=bvec.ap())
            ones1 = cpool.tile([1, D], f32)
            nc.vector.memset(ones1[:], 1.0)

            # ---- gather indices (one DMA) ----
            gidx_t = mpool.tile([128, n_idx_cols], i16)
            nc.sync.dma_start(out=gidx_t[:], in_=gidx.ap())

            # precompute static offsets
            idx_off = {}
            off = 0
            for sb in range(N_SB):
                for t in range(N_SUBT):
                    idx_off[(sb, t)] = off
                    off += nchunks_sbt[sb][t] * 128 // 16

            gmax = [max(nchunks_sbt[sb][t] for sb in range(N_SB)) for t in range(N_SUBT)]
            _starts = []
            for _g in range(0, NBLK, 7):
                _e = _g + 7
                _p0 = 0
                for _b in range(_g):
                    _p0 += len(chunk_plan[_b])
                _p1 = _p0
                for _b in range(_g, min(_e, NBLK)):
                    _p1 += len(chunk_plan[_b])
                _starts.append(_p1 - _p0)
            ghw_max = max(_starts)
            pos_of = {}
            _p = 0
            for _b in range(NBLK):
                pos_of[_b] = _p
                _p += len(chunk_plan[_b])

            PIECE = 16  # chunks per gather instruction (2048 idxs)
            swdge_i = 0
            for sb in range(N_SB):
                # ---- gather instructions for this superblock, in pieces ----
                gtiles = []
                for t in range(N_SUBT):
                    nck = nchunks_sbt[sb][t]
                    gt = gpool.tile([128, gmax[t] * 128], bf16, tag=f"g{t}")
                    base = t * SUBT_ROWS
                    rows = min(SUBT_ROWS, NUM_NODES - base)
                    io = idx_off[(sb, t)]
                    for p0 in range(0, nck, PIECE):
                        pk = min(PIECE, nck - p0)
                        n = pk * 128
                        nc.gpsimd.dma_gather(
                            out_ap=gt[:, p0 * 128 : (p0 + pk) * 128].rearrange(
                                "p (c r) -> p c r", r=128
                            ),
                            in_ap=xbf.ap()[base : base + rows, :],
                            idxs_ap=gidx_t[
                                :, io + p0 * 8 : io + (p0 + pk) * 8
                            ],
                            num_idxs=n,
                            num_idxs_reg=n,
                            elem_size=D,
                            single_packet=False,
                            queue_num=swdge_i % 4,
                        )
                        swdge_i += 1
                    gtiles.append(gt)

                for half in range(2):
                    g0 = sb * SB_BLOCKS + half * 7
                    p0 = pos_of[g0]
                    p1 = pos_of[g0 + 7] if g0 + 7 < NBLK else NC_TOT
                    nchv = p1 - p0
                    ghw = ohpool.tile([128, ghw_max * 128], bf16, tag="ghw")
                    nc.scalar.dma_start(
                        out=ghw[:, : nchv * 128],
                        in_=ohw_meta.ap()[:, p0 * 128 : p1 * 128],
                    )
                    ghe = ohpool.tile([128, ghw_max * NUM_REL], bf16, tag="ghe")
                    nc.scalar.dma_start(
                        out=ghe[:, : nchv * NUM_REL],
                        in_=ohe_meta.ap()[:, p0 * NUM_REL : p1 * NUM_REL],
                    )
                    x7 = bpool.tile([128, 7 * 128], f32, tag="x7")
                    nc.sync.dma_start(
                        out=x7[:],
                        in_=x_shard.ap()[g0 * 128 : (g0 + 7) * 128, :].rearrange(
                            "(c p) f -> p c f", p=128
                        ),
                    )
                    o7 = bpool.tile([128, 7 * 128], f32, tag="o7")
                    for bi in range(7):
                        blk = g0 + bi
                        plan = chunk_plan[blk]
                        assert plan, f"block {blk} has no chunks"
                        nchunk = len(plan)
                        bpos = pos_of[blk] - p0
                        sT = pspool.tile([128, 128], f32, tag="sT")
                        cT = pspool.tile([NUM_REL, 128], f32, tag="cT")
                        for ci, (t, slot, gchunk) in enumerate(plan):
                            ohw = ghw[:, (bpos + ci) * 128 : (bpos + ci + 1) * 128]
                            xg = gtiles[t][:, slot * 128 : (slot + 1) * 128]
                            nc.tensor.matmul(
                                out=sT[:], lhsT=xg, rhs=ohw,
                                start=(ci == 0), stop=(ci == nchunk - 1),
                            )
                        for ci, (t, slot, gchunk) in enumerate(plan):
                            ohw = ghw[:, (bpos + ci) * 128 : (bpos + ci + 1) * 128]
                            ohe = ghe[
                                :,
                                (bpos + ci) * NUM_REL : (bpos + ci + 1) * NUM_REL,
                            ]
                            nc.tensor.matmul(
                                out=cT[:], lhsT=ohe, rhs=ohw,
                                start=(ci == 0), stop=(ci == nchunk - 1),
                            )

                        # ---- block epilogue ----
                        sT_sb = bpool.tile([128, 128], bf16, tag="sTsb")
                        nc.vector.tensor_copy(out=sT_sb[:], in_=sT[:])
                        cT_sb = bpool.tile([NUM_REL, 128], bf16, tag="cTsb")
                        nc.vector.tensor_copy(out=cT_sb[:], in_=cT[:])
                        x_blk = x7[:, bi * 128 : (bi + 1) * 128]
                        xT_ps = psopool.tile([128, 128], f32, tag="xT")
                        nc.tensor.transpose(
                            out=xT_ps[:], in_=x_blk, identity=ident[:]
                        )
                        xT_sb = bpool.tile([128, 128], f32, tag="xTsb")
                        nc.vector.tensor_copy(out=xT_sb[:], in_=xT_ps[:])

                        acc = psopool.tile([128, D], f32, tag="acc")
                        nc.tensor.matmul(
                            out=acc[:], lhsT=sT_sb[:], rhs=wmsg_b[:],
                            start=True, stop=False,
                        )
                        nc.tensor.matmul(
                            out=acc[:], lhsT=cT_sb[:], rhs=rb_b[:],
                            start=False, stop=False,
                        )
                        nc.tensor.matmul(
                            out=acc[:], lhsT=xT_sb[:], rhs=wself_f[:],
                            start=False, stop=False,
                        )
                        nc.tensor.matmul(
                            out=acc[:], lhsT=ones1[:], rhs=b_row[:],
                            start=False, stop=True,
                        )
                        nc.scalar.activation(
                            out=o7[:, bi * 128 : (bi + 1) * 128],
                            in_=acc[:],
                            func=mybir.ActivationFunctionType.Relu,
                        )
                    nc.sync.dma_start(
                        out=out_d.ap()[g0 * 128 : (g0 + 7) * 128, :].rearrange(
                            "(c p) f -> p c f", p=128
                        ),
                        in_=o7[:],
                    )

    nc.compile()
    return nc


def _prep(inputs):
    """Host-side sharding/layout. Returns (in_maps, static_key, layout)."""
    x = np.ascontiguousarray(np.asarray(inputs["x"], dtype=np.float32))
    source = np.asarray(inputs["source"]).astype(np.int64)
    target = np.asarray(inputs["target"]).astype(np.int64)
    edge_type = np.asarray(inputs["edge_type"]).astype(np.int64)
    ew = np.asarray(inputs["edge_weights"], dtype=np.float32)
    w_msg = np.ascontiguousarray(np.asarray(inputs["W_msg"], dtype=np.float32))
    rel_bias = np.ascontiguousarray(np.asarray(inputs["rel_bias"], dtype=np.float32))
    w_self = np.ascontiguousarray(np.asarray(inputs["W_self"], dtype=np.float32))
    b = np.asarray(inputs["b"], dtype=np.float32).reshape(1, D)

    n = x.shape[0]
    assert n == NUM_NODES

    xbf = x.astype(ml_dtypes.bfloat16)

    core = target // NODES_PER_CORE
    tgt_local = target - core * NODES_PER_CORE
    blk = tgt_local >> 7
    tgt_in_blk = tgt_local & 127
    subt = source // SUBT_ROWS
    src_local = source - subt * SUBT_ROWS

    # per (core, blk, subtable) edge index lists
    # order edges by (core, blk, subt) with a stable sort
    key = ((core * NBLK + blk) * N_SUBT + subt).astype(np.int64)
    order = np.argsort(key, kind="stable")
    key_s = key[order]
    # group boundaries
    uniq, starts = np.unique(key_s, return_index=True)
    counts = np.diff(np.append(starts, key_s.shape[0]))

    cnt = np.zeros((N_CORES, NBLK, N_SUBT), dtype=np.int64)
    ci = uniq // (NBLK * N_SUBT)
    bi = (uniq // N_SUBT) % NBLK
    ti = uniq % N_SUBT
    cnt[ci, bi, ti] = counts

    # static chunk capacity per (blk, subtable): max over cores
    c_bt = np.ceil(cnt.max(axis=0) / 128).astype(np.int64)  # (NBLK, N_SUBT)
    # ensure every block has at least one chunk
    empty = c_bt.sum(axis=1) == 0
    c_bt[empty, 0] = 1

    nchunks_sbt = [
        [int(c_bt[sb * SB_BLOCKS : (sb + 1) * SB_BLOCKS, t].sum()) for t in range(N_SUBT)]
        for sb in range(N_SB)
    ]
    NC_TOT = int(c_bt.sum())

    # global chunk ids: order is (sb, t, blk-within-sb, chunk)
    gchunk_of = np.zeros((NBLK, N_SUBT), dtype=np.int64)  # first chunk id
    slot_of = np.zeros((NBLK, N_SUBT), dtype=np.int64)    # first slot in (sb,t) tile
    g = 0
    for sb in range(N_SB):
        for t in range(N_SUBT):
            s = 0
            for bi2 in range(SB_BLOCKS):
                bb = sb * SB_BLOCKS + bi2
                gchunk_of[bb, t] = g
                slot_of[bb, t] = s
                g += int(c_bt[bb, t])
                s += int(c_bt[bb, t])
    assert g == NC_TOT

    chunk_plan = []
    for bb in range(NBLK):
        plan = []
        for t in range(N_SUBT):
            for c in range(int(c_bt[bb, t])):
                plan.append((t, int(slot_of[bb, t] + c), int(gchunk_of[bb, t] + c)))
        chunk_plan.append(plan)

    # position of each block's chunk run in the (block-major) onehot layout
    pos_of_blk = np.zeros(NBLK, dtype=np.int64)
    p = 0
    for bb in range(NBLK):
        pos_of_blk[bb] = p
        p += len(chunk_plan[bb])
    # gchunk -> block-major position
    pos_of_gchunk = np.zeros(NC_TOT, dtype=np.int64)
    for bb in range(NBLK):
        for i, (_t, _s, g2) in enumerate(chunk_plan[bb]):
            pos_of_gchunk[g2] = pos_of_blk[bb] + i

    n_idx_cols = sum(nc_ * 128 // 16 for row in nchunks_sbt for nc_ in row)

    # build per-core tensors
    in_maps = []
    # offsets of edge groups in the sorted edge array, per core
    start_of = {}
    for u, s0, c0 in zip(uniq, starts, counts):
        start_of[int(u)] = (int(s0), int(c0))

    for c in range(N_CORES):
        gidx = np.zeros((128, n_idx_cols), dtype=np.int16)
        ohw_m = np.zeros((128, NC_TOT * 128), dtype=ml_dtypes.bfloat16)
        ohe_m = np.zeros((128, NC_TOT * NUM_REL), dtype=ml_dtypes.bfloat16)

        icol = 0
        for sb in range(N_SB):
            for t in range(N_SUBT):
                nck = nchunks_sbt[sb][t]
                if nck == 0:
                    continue
                nslots = nck * 128
                idxs = np.zeros(nslots, dtype=np.int16)
                for bi2 in range(SB_BLOCKS):
                    bb = sb * SB_BLOCKS + bi2
                    k = (c * NBLK + bb) * N_SUBT + t
                    s0, n_e = start_of.get(k, (0, 0))
                    sl0 = int(slot_of[bb, t]) * 128 - int(slot_of[sb * SB_BLOCKS, t]) * 128
                    g0 = int(gchunk_of[bb, t])
                    if n_e:
                        eids = order[s0 : s0 + n_e]
                        idxs[sl0 : sl0 + n_e] = src_local[eids].astype(np.int16)
                        # meta: chunk-major [128 partitions]
                        for cc in range(int(c_bt[bb, t])):
                            lo = cc * 128
                            hi = min(n_e, lo + 128)
                            if hi <= lo:
                                break
                            ecol = eids[lo:hi]
                            gc = g0 + cc
                            npart = hi - lo
                            pos = int(pos_of_gchunk[gc])
                            parts = np.arange(npart)
                            ohw_m[parts, pos * 128 + tgt_in_blk[ecol]] = ew[
                                ecol
                            ].astype(ml_dtypes.bfloat16)
                            ohe_m[parts, pos * NUM_REL + edge_type[ecol]] = 1.0
                # wrap idxs: element j -> partition j%16, col j//16; replicate x8
                wrapped = idxs.reshape(nslots // 16, 16).T  # (16, nslots/16)
                gidx[:, icol : icol + nslots // 16] = np.tile(wrapped, (8, 1))
                icol += nslots // 16
        assert icol == n_idx_cols

        xs = np.zeros((NODES_PER_CORE, D), dtype=np.float32)
        lo = c * NODES_PER_CORE
        hi = min(lo + NODES_PER_CORE, NUM_NODES)
        xs[: hi - lo] = x[lo:hi]

        in_maps.append(
            {
                "xbf": xbf,
                "x_shard": xs,
                "w_msg": w_msg,
                "w_self": w_self,
                "rel_bias": rel_bias,
                "bvec": b,
                "gidx": gidx,
                "ohw_meta": ohw_m,
                "ohe_meta": ohe_m,
            }
        )

    static_key = tuple(c_bt.flatten().tolist())
    return in_maps, static_key, (nchunks_sbt, chunk_plan)


def kernel(**inputs) -> np.ndarray:
    from concourse import bass_utils

    in_maps, static_key, (nchunks_sbt, chunk_plan) = _prep(inputs)

    nc = _kernel_cache.get(static_key)
    if nc is None:
        nc = _build_and_compile(static_key, nchunks_sbt, chunk_plan)
        _kernel_cache[static_key] = nc

    res = bass_utils.run_bass_kernel_spmd(
        nc, in_maps, core_ids=list(range(N_CORES))
    )
    parts = [res.results[c]["out"] for c in range(N_CORES)]
    full = np.concatenate(parts, axis=0)[:NUM_NODES]
    return full.astype(np.float32)
